# revision 22
# baseline (speedup 1.0000x reference)
"""InternLM2 decoder layer on 8 trn2 NeuronCores, tensor-parallel (bass/Tile).

Self-contained: hardcodes shapes/sharding. Host shards + pre-tiles weights
(bf16, RMSNorm gammas folded into consuming matmul weights), device computes
the layer, host reassembles the output.

Per-core sharding: q-heads 4c..4c+3 + kv-head c (GQA groups align), wo/w2
row-sharded, w1/w3 col-sharded. Collectives are chunked for overlap:
AG1/AG2 split into 4 hid-quarters (k-chains start on quarter 0), RS1/RS2
split into 4 token-chunks fired right after the chunk's producer. RS chunk
j hands core c rows [64c:64c+64], so core c owns 64-token pieces
{512j + 64c : j=0..3} for norm2 / h / final residual / output; the host
slices x2_own and reassembles out accordingly.

Dataflow: norm1(own 256 tok) -> AG1 quarters -> per-512-chunk
QKV/rope/attention/wo -> RS1_j per chunk -> fused resid+norm2 per piece ->
AG2 quarters -> MLP in 512-token quarters (w1/w3/w2 streamed) -> RS2_q per
quarter -> final residual strips (overlapped). Scores computed transposed
[s, t]; softmax denominator via ones-matmul, reciprocal on DVE,
partition-broadcast via rank-1 PE matmul, causal mask via precomputed bf16
mask multiply on DVE.
"""
import sys
import numpy as np
import ml_dtypes

sys.path.insert(0, "/opt/trn_rl_repo")

HID, H, K, D, INTER, T = 4096, 32, 8, 128, 14336, 2048
EPS, THETA = 1e-5, 1000000.0
NC = 8                 # cores
QH = H // NC           # q heads per core = 4
JD = QH * D            # per-core attn out dim = 512
IS = INTER // NC       # inter shard = 1792
TOK = T // NC          # owned tokens per core = 256
CH = 512               # token chunk for compute loops
NCH = T // CH          # 4
KB_ = HID // 128       # 32 k-tiles
KBQ = KB_ // 4         # 8 k-tiles per hid quarter
IT_ = IS // 128        # 14 i-tiles
SCALE = 1.0 / np.sqrt(D)

bf16 = ml_dtypes.bfloat16

_compiled = None


def _build():
    from contextlib import ExitStack
    import concourse.bacc as bacc
    import concourse.bass as bass
    import concourse.tile as tile
    from concourse import mybir

    f32 = mybir.dt.float32
    bf = mybir.dt.bfloat16
    AF = mybir.ActivationFunctionType
    PSUM = bass.MemorySpace.PSUM

    nc = bacc.Bacc("TRN2", target_bir_lowering=False, debug=False, num_devices=NC)

    # ---- I/O (per-core shapes; weights pre-tiled on host) ----
    x_own = nc.dram_tensor("x_own", [TOK, HID], f32, kind="ExternalInput")
    x2_own = nc.dram_tensor("x2_own", [4, 64, HID], f32, kind="ExternalInput")
    cosT = nc.dram_tensor("cosT", [D // 2, T], bf, kind="ExternalInput")
    sinT = nc.dram_tensor("sinT", [D // 2, T], bf, kind="ExternalInput")
    ident = nc.dram_tensor("ident", [128, 128], bf, kind="ExternalInput")
    masksI = nc.dram_tensor("masksI", [128, 4, CH], bf, kind="ExternalInput")
    wqkvR = nc.dram_tensor("wqkvR", [128, KB_, JD + 2 * D], bf, kind="ExternalInput")
    woR = nc.dram_tensor("woR", [128, QH, HID], bf, kind="ExternalInput")
    w1R = nc.dram_tensor("w1R", [IT_, 128, KB_, 128], bf, kind="ExternalInput")
    w3R = nc.dram_tensor("w3R", [IT_, 128, KB_, 128], bf, kind="ExternalInput")
    w2R = nc.dram_tensor("w2R", [128, IT_, HID], bf, kind="ExternalInput")
    out_own = nc.dram_tensor("out_own", [4, 64, HID], f32, kind="ExternalOutput")

    # ---- internal DRAM (collective bounce + h spill) ----
    ag1_in = [nc.dram_tensor(f"ag1_in{q}", [128, KBQ, TOK], bf, kind="Internal")
              for q in range(4)]
    ag1_out = [nc.dram_tensor(f"ag1_out{q}", [NC, 128, KBQ, TOK], bf,
                              kind="Internal", addr_space="Shared")
               for q in range(4)]
    rs1_in = [nc.dram_tensor(f"rs1_in{j}", [CH, HID], bf, kind="Internal")
              for j in range(4)]
    rs1_out = [nc.dram_tensor(f"rs1_out{j}", [64, HID], bf, kind="Internal")
               for j in range(4)]
    ag2_in = [nc.dram_tensor(f"ag2_in{j}", [128, KB_, 64], bf, kind="Internal")
              for j in range(4)]
    ag2_out = [nc.dram_tensor(f"ag2_out{j}", [NC, 128, KB_, 64], bf,
                              kind="Internal", addr_space="Shared")
               for j in range(4)]
    rs2_in = [nc.dram_tensor(f"rs2_in{j}", [CH, HID], bf, kind="Internal")
              for j in range(4)]
    rs2_out = [nc.dram_tensor(f"rs2_out{j}", [64, HID], bf, kind="Internal")
               for j in range(4)]
    h_spill = nc.dram_tensor("h_spill", [4, 64, HID], bf, kind="Internal")

    RG = [list(range(NC))]

    def allgather(in_t, out_t):
        nc.gpsimd.collective_compute(
            "AllGather", mybir.AluOpType.bypass, replica_groups=RG,
            ins=[in_t.ap()], outs=[out_t.ap()])

    def reducescatter(in_ap, out_t):
        nc.gpsimd.collective_compute(
            "ReduceScatter", mybir.AluOpType.add, replica_groups=RG,
            ins=[in_ap], outs=[out_t.ap()])

    with tile.TileContext(nc) as tc, ExitStack() as top:
        const = top.enter_context(tc.tile_pool(name="const", bufs=1))
        ident_sb = const.tile([128, 128], bf)
        ones_sb = const.tile([128, 1], bf)
        nc.vector.memset(ones_sb[:], 1.0)
        eps_sb = const.tile([128, 1], f32)
        nc.vector.memset(eps_sb[:], EPS)
        masks_sb = const.tile([128, 4, CH], bf)
        cos_sb = const.tile([D // 2, T], bf)
        sin_sb = const.tile([D // 2, T], bf)

        # attention-weight pool opened before phase-1 pools (stack order);
        # its DMAs are emitted after the x loads so norm1 starts immediately.
        # Explicitly closed after phase 2 so MLP-phase SBUF fits.
        wctx = tc.tile_pool(name="wattn", bufs=1)
        wpool = wctx.__enter__()
        wqkv_sb = wpool.tile([128, KB_, JD + 2 * D], bf)
        wo_sb = wpool.tile([128, QH, HID], bf)

        ph1 = ExitStack()
        pool1 = ph1.enter_context(tc.tile_pool(name="norm1", bufs=1))
        stg1 = ph1.enter_context(tc.tile_pool(name="n1stage", bufs=1))
        ps1 = ph1.enter_context(tc.tile_pool(name="n1ps", bufs=4, space=PSUM))
        xts = []
        for b in range(TOK // 128):
            xt = pool1.tile([128, HID], f32, tag=f"xt{b}", name=f"xt{b}")
            nc.sync.dma_start(xt[:], x_own.ap()[b * 128:(b + 1) * 128, :])
            xts.append(xt)
        nc.sync.dma_start(ident_sb[:], ident.ap())
        nc.sync.dma_start(masks_sb[:], masksI.ap())
        nc.sync.dma_start(cos_sb[:], cosT.ap())
        nc.sync.dma_start(sin_sb[:], sinT.ap())
        nc.sync.dma_start(wqkv_sb[:], wqkvR.ap())
        nc.sync.dma_start(wo_sb[:], woR.ap())

        # rms-normalize a [p, HID] f32 SBUF tile in place -> bf16 xn tile
        def rms_norm(pool, src, p, tagp):
            sq = pool.tile([p, HID], bf, tag=tagp + "sq", name=tagp + "sq")
            ssq = pool.tile([p, 1], f32, tag=tagp + "ssq", name=tagp + "ssq")
            nc.scalar.activation(sq[:], src, AF.Square, accum_out=ssq[:])
            rms = pool.tile([p, 1], f32, tag=tagp + "rm", name=tagp + "rm")
            nc.scalar.activation(rms[:], ssq[:], AF.Sqrt,
                                 scale=1.0 / HID, bias=eps_sb[0:p, :])
            rinv = pool.tile([p, 1], f32, tag=tagp + "ri", name=tagp + "ri")
            nc.vector.reciprocal(rinv[:], rms[:])
            xn = pool.tile([p, HID], bf, tag=tagp + "xn", name=tagp + "xn")
            nc.vector.tensor_scalar_mul(xn[:], src, rinv[:])
            return xn

        # ================= phase 1: norm1 + AG1 (hid quarters) ===============
        with ph1, nc.named_scope("norm1"):
            stage1 = [stg1.tile([128, KBQ, TOK], bf, tag=f"s1{q}",
                                name=f"s1{q}") for q in range(4)]
            xns = [rms_norm(pool1, xts[b][:], 128, f"n1b{b}")
                   for b in range(TOK // 128)]
            for q in range(4):
                for b in range(TOK // 128):
                    for a in range(KBQ):
                        kb = q * KBQ + a
                        tp = ps1.tile([128, 128], bf, tag="tp", name="tp")
                        nc.tensor.transpose(
                            tp[:], xns[b][:, kb * 128:(kb + 1) * 128], ident_sb[:])
                        nc.vector.tensor_copy(
                            stage1[q][:, a, b * 128:(b + 1) * 128], tp[:])
                nc.sync.dma_start(ag1_in[q].ap(), stage1[q][:])
                allgather(ag1_in[q], ag1_out[q])

        # ============ phase 2: QKV + attention + wo, chunked RS1 ============
        with ExitStack() as ph:
            kv_pool = ph.enter_context(tc.tile_pool(name="kv", bufs=1))
            kT_sb = kv_pool.tile([128, T], bf)             # roped K, [d, s]
            v_sb = kv_pool.tile([128, T // 128, D], bf)    # [s-part, s-tile, d]

            xc_pool = ph.enter_context(tc.tile_pool(name="attnxc", bufs=1))
            ap_ = ph.enter_context(tc.tile_pool(name="attn", bufs=2))
            ps_acc = ph.enter_context(tc.tile_pool(name="accps", bufs=2, space=PSUM))
            ps_sc = ph.enter_context(tc.tile_pool(name="scps", bufs=2, space=PSUM))
            ps_pv = ph.enter_context(tc.tile_pool(name="pvps", bufs=2, space=PSUM))
            ps_sm = ph.enter_context(tc.tile_pool(name="smps", bufs=1, space=PSUM))
            n2pool = ph.enter_context(tc.tile_pool(name="norm2", bufs=1))
            n2ps = ph.enter_context(tc.tile_pool(name="n2ps", bufs=1, space=PSUM))

            # fused h = x2 + rs1_out[jp], rms-norm, transpose, AG per 64-piece.
            # Strip-processed to keep SBUF small; h stored bf16.
            def norm2_piece(jp, gate=None, ring=None):
                ring_ = ring if ring is not None else nc.sync
                def _gate(inst):
                    if gate is not None:
                        bass._add_dep_helper(
                            inst.ins, gate.ins, sync=False,
                            reason="norm2 ordered after next chunk's rope")
                with nc.named_scope(f"norm2_{jp}"):
                    hts, ssqs = [], []
                    for s in range(4):
                        cs = slice(s * 1024, (s + 1) * 1024)
                        xts = n2pool.tile([64, 1024], f32, tag="hxt", name="hxt")
                        _gate(ring_.dma_start(xts[:], x2_own.ap()[jp][:, cs]))
                        rts = n2pool.tile([64, 1024], bf, tag="hrt", name="hrt")
                        _gate(ring_.dma_start(rts[:], rs1_out[jp].ap()[:, cs]))
                        ht = n2pool.tile([64, 1024], bf, tag=f"ht{s}",
                                         name=f"ht{s}")
                        _gate(nc.vector.tensor_add(ht[:], xts[:], rts[:]))
                        ring_.dma_start(h_spill.ap()[jp][:, cs], ht[:])
                        sq = n2pool.tile([64, 1024], bf, tag="hsq", name="hsq")
                        ssq = n2pool.tile([64, 1], f32, tag=f"hssq{s}",
                                          name=f"hssq{s}")
                        nc.scalar.activation(sq[:], ht[:], AF.Square,
                                             accum_out=ssq[:])
                        hts.append(ht)
                        ssqs.append(ssq)
                    tot = n2pool.tile([64, 1], f32, tag="htot", name="htot")
                    nc.vector.tensor_add(tot[:], ssqs[0][:], ssqs[1][:])
                    nc.vector.tensor_add(tot[:], tot[:], ssqs[2][:])
                    nc.vector.tensor_add(tot[:], tot[:], ssqs[3][:])
                    rms = n2pool.tile([64, 1], f32, tag="hrm", name="hrm")
                    nc.scalar.activation(rms[:], tot[:], AF.Sqrt,
                                         scale=1.0 / HID, bias=eps_sb[0:64, :])
                    rinv = n2pool.tile([64, 1], f32, tag="hri", name="hri")
                    nc.vector.reciprocal(rinv[:], rms[:])
                    stg = n2pool.tile([128, KB_, 64], bf, tag="hstg",
                                      name="hstg")
                    for s in range(4):
                        xn = n2pool.tile([64, 1024], bf, tag="hxn", name="hxn")
                        nc.vector.tensor_scalar_mul(xn[:], hts[s][:], rinv[:])
                        for a in range(8):
                            kb = 8 * s + a
                            tp = n2ps.tile([128, 64], bf, tag="n2tp",
                                           name="n2tp")
                            nc.tensor.transpose(
                                tp[:], xn[:, a * 128:(a + 1) * 128],
                                ident_sb[0:64, 0:64])
                            nc.vector.tensor_copy(stg[:, kb, :], tp[:])
                    ring_.dma_start(ag2_in[jp].ap(), stg[:])
                    allgather(ag2_in[jp], ag2_out[jp])

            def rope(dst, src, t0):
                c = cos_sb[:, t0:t0 + CH]
                s = sin_sb[:, t0:t0 + CH]
                t1 = ap_.tile([64, CH], f32, tag="rp1")
                t2 = ap_.tile([64, CH], f32, tag="rp2")
                nc.vector.tensor_mul(t1[:], src[0:64, :], c)
                nc.vector.tensor_mul(t2[:], src[64:128, :], s)
                nc.vector.tensor_sub(dst[0:64, :], t1[:], t2[:])
                nc.vector.tensor_mul(t1[:], src[64:128, :], c)
                nc.vector.tensor_mul(t2[:], src[0:64, :], s)
                return nc.vector.tensor_add(dst[64:128, :], t1[:], t2[:])

            for j in range(NCH):
                t0 = j * CH
                with nc.named_scope(f"attn{j}"):
                    xc = xc_pool.tile([128, KB_, CH], bf, tag="xc")
                    for q in range(4):
                        for ci, c in enumerate((2 * j, 2 * j + 1)):
                            nc.sync.dma_start(
                                xc[:, q * KBQ:(q + 1) * KBQ,
                                   ci * 256:(ci + 1) * 256],
                                ag1_out[q].ap()[c])
                    qT = ap_.tile([128, QH, CH], bf, tag="qT")
                    for m in range(6):
                        acc = ps_acc.tile([128, CH], f32, tag="acc")
                        for kb in range(KB_):
                            nc.tensor.matmul(
                                acc[:],
                                wqkv_sb[:, kb, m * 128:(m + 1) * 128],
                                xc[:, kb, :],
                                start=(kb == 0), stop=(kb == KB_ - 1))
                        if m < QH:
                            rope(qT[:, m, :], acc, t0)
                        elif m == QH:
                            last_rope = rope(kT_sb[:, t0:t0 + CH], acc, t0)
                        else:
                            vb = ap_.tile([128, CH], bf, tag="vb")
                            nc.vector.tensor_copy(vb[:], acc[:])
                            for sb_ in range(CH // 128):
                                tp = ps_sc.tile([128, 128], bf, tag="sc")
                                nc.tensor.transpose(
                                    tp[:], vb[:, sb_ * 128:(sb_ + 1) * 128],
                                    ident_sb[:])
                                nc.vector.tensor_copy(
                                    v_sb[:, 4 * j + sb_, :], tp[:])
                    if j >= 1:
                        norm2_piece(j - 1, gate=last_rope)

                    aoT = ap_.tile([128, QH, CH], bf, tag="aoT")
                    ns = 4 * j + 4
                    for hq in range(QH):
                        pv = ps_pv.tile([128, CH], f32, tag="pv")
                        den = ps_sm.tile([1, CH], f32, tag="den")
                        for si in range(ns):
                            sc = ps_sc.tile([128, CH], f32, tag="sc")
                            nc.tensor.matmul(
                                sc[:], kT_sb[:, si * 128:(si + 1) * 128],
                                qT[:, hq, :], start=True, stop=True)
                            pT = ap_.tile([128, CH], bf, tag="pT")
                            nc.scalar.activation(pT[:], sc[:], AF.Exp, scale=SCALE)
                            if si >= 4 * j:          # diagonal: zero s > t
                                pm = ap_.tile([128, CH], bf, tag="pm")
                                nc.vector.tensor_mul(
                                    pm[:], pT[:], masks_sb[:, si - 4 * j, :])
                                pT = pm
                            nc.tensor.matmul(pv[:], v_sb[:, si, :], pT[:],
                                             start=(si == 0), stop=(si == ns - 1))
                            nc.tensor.matmul(den[:], ones_sb[:], pT[:],
                                             start=(si == 0), stop=(si == ns - 1))
                        rec = ap_.tile([1, CH], f32, tag="rec")
                        nc.vector.reciprocal(rec[:], den[:])
                        rcs = ap_.tile([128, CH], f32, tag="rcs")
                        nc.gpsimd.partition_broadcast(rcs[:], rec[:])
                        nc.vector.tensor_mul(aoT[:, hq, :], pv[:], rcs[:])

                    # wo: out[t, hid] rows t0+128m, K=512 over heads
                    for m in range(CH // 128):
                        for nh in range(8):
                            acc = ps_acc.tile([128, 512], f32, tag="acc")
                            for kb in range(QH):
                                nc.tensor.matmul(
                                    acc[:],
                                    aoT[:, kb, m * 128:(m + 1) * 128],
                                    wo_sb[:, kb, nh * 512:(nh + 1) * 512],
                                    start=(kb == 0), stop=(kb == QH - 1))
                            ob = ap_.tile([128, 512], bf, tag="ob")
                            nc.vector.tensor_copy(ob[:], acc[:])
                            nc.sync.dma_start(
                                rs1_in[j].ap()[m * 128:(m + 1) * 128,
                                               nh * 512:(nh + 1) * 512], ob[:])
                reducescatter(rs1_in[j].ap(), rs1_out[j])
            norm2_piece(NCH - 1, ring=nc.scalar)
        wctx.__exit__(None, None, None)

        # ============== phase 4: MLP in 512-token quarters ==============
        with ExitStack() as ph:
            mxc = ph.enter_context(tc.tile_pool(name="mlpxc", bufs=2))
            mact = ph.enter_context(tc.tile_pool(name="mlpact", bufs=2))
            mw = ph.enter_context(tc.tile_pool(name="mlpw", bufs=2))
            mw2 = ph.enter_context(tc.tile_pool(name="mlpw2", bufs=2))
            msc = ph.enter_context(tc.tile_pool(name="mlpsc", bufs=3))
            fpool = ph.enter_context(tc.tile_pool(name="fin", bufs=2))
            ps_g = ph.enter_context(tc.tile_pool(name="gups", bufs=3, space=PSUM))
            ps_d = ph.enter_context(tc.tile_pool(name="dps", bufs=2, space=PSUM))

            def fin_piece(j, gate=None):
                def _gate(inst):
                    if gate is not None:
                        bass._add_dep_helper(
                            inst.ins, gate.ins, sync=False,
                            reason="fin ordered after next quarter's g/u")
                with nc.named_scope(f"fin{j}"):
                    for s in range(4):
                        cs = slice(s * 1024, (s + 1) * 1024)
                        ht = fpool.tile([64, 1024], bf, tag="fht")
                        _gate(nc.sync.dma_start(ht[:], h_spill.ap()[j][:, cs]))
                        rt = fpool.tile([64, 1024], bf, tag="frt")
                        _gate(nc.sync.dma_start(rt[:], rs2_out[j].ap()[:, cs]))
                        ot = fpool.tile([64, 1024], f32, tag="fot")
                        _gate(nc.vector.tensor_add(ot[:], ht[:], rt[:]))
                        nc.sync.dma_start(out_own.ap()[j][:, cs], ot[:])

            for q in range(NCH):
                t0 = q * CH
                with nc.named_scope(f"mlp{q}"):
                    xc = mxc.tile([128, KB_, CH], bf, tag="xcq")
                    for qh in range(4):
                        for c in range(NC):
                            nc.sync.dma_start(
                                xc[:, qh * KBQ:(qh + 1) * KBQ,
                                   c * 64:(c + 1) * 64],
                                ag2_out[q].ap()[c][:,
                                    qh * KBQ:(qh + 1) * KBQ, :])
                    actT = mact.tile([128, IT_, CH], bf, tag="actT")
                    for it in range(IT_):
                        w1t = mw.tile([128, KB_, 128], bf, tag="w1t")
                        w3t = mw.tile([128, KB_, 128], bf, tag="w3t")
                        nc.sync.dma_start(w1t[:], w1R.ap()[it])
                        nc.sync.dma_start(w3t[:], w3R.ap()[it])
                        gp = ps_g.tile([128, CH], f32, tag="g")
                        up = ps_g.tile([128, CH], f32, tag="u")
                        for kb in range(KB_):
                            nc.tensor.matmul(gp[:], w1t[:, kb, :], xc[:, kb, :],
                                             start=(kb == 0), stop=(kb == KB_ - 1))
                        for kb in range(KB_):
                            nc.tensor.matmul(up[:], w3t[:, kb, :], xc[:, kb, :],
                                             start=(kb == 0), stop=(kb == KB_ - 1))
                        sg = msc.tile([128, CH], f32, tag="sg")
                        nc.scalar.activation(sg[:], gp[:], AF.Silu)
                        last_mul = nc.vector.tensor_mul(
                            actT[:, it, :], sg[:], up[:])
                    if q >= 1:
                        fin_piece(q - 1, gate=last_mul)
                    # down-proj: stream w2 col slices, contract over it
                    for s8 in range(8):
                        w2s = mw2.tile([128, IT_, 512], bf, tag="w2s")
                        nc.sync.dma_start(
                            w2s[:], w2R.ap()[:, :, s8 * 512:(s8 + 1) * 512])
                        for m in range(CH // 128):
                            acc = ps_d.tile([128, 512], f32, tag="d")
                            for it in range(IT_):
                                nc.tensor.matmul(
                                    acc[:], actT[:, it, m * 128:(m + 1) * 128],
                                    w2s[:, it, :],
                                    start=(it == 0), stop=(it == IT_ - 1))
                            ob = msc.tile([128, 512], bf, tag="ob")
                            nc.vector.tensor_copy(ob[:], acc[:])
                            nc.sync.dma_start(
                                rs2_in[q].ap()[m * 128:(m + 1) * 128,
                                               s8 * 512:(s8 + 1) * 512], ob[:])
                reducescatter(rs2_in[q].ap(), rs2_out[q])
            fin_piece(NCH - 1)

    nc.compile()
    return nc


def _get_compiled():
    global _compiled
    if _compiled is None:
        _compiled = _build()
    return _compiled


def _prep_inputs(inputs):
    x = np.asarray(inputs["hidden_states"], np.float32)
    pos = np.asarray(inputs["position_ids"]).astype(np.float32)
    wqkv = np.asarray(inputs["wqkv"], np.float32)
    wo = np.asarray(inputs["wo"], np.float32)
    w1 = np.asarray(inputs["w1"], np.float32)
    w3 = np.asarray(inputs["w3"], np.float32)
    w2 = np.asarray(inputs["w2"], np.float32)
    anw = np.asarray(inputs["attn_norm_w"], np.float32)
    fnw = np.asarray(inputs["ffn_norm_w"], np.float32)

    inv_freq = 1.0 / (THETA ** (np.arange(0, D, 2, dtype=np.float32) / D))
    freqs = pos[:, None] * inv_freq
    cosT_np = np.ascontiguousarray(np.cos(freqs).T.astype(bf16))
    sinT_np = np.ascontiguousarray(np.sin(freqs).T.astype(bf16))
    ident_np = np.ascontiguousarray(np.eye(128, dtype=bf16))

    # causal masks for diagonal tiles: masks[p, r, f] = (f >= 128*r + p)
    p_ = np.arange(128)[:, None, None]
    r_ = np.arange(4)[None, :, None]
    f_ = np.arange(CH)[None, None, :]
    masks_np = np.ascontiguousarray((f_ >= 128 * r_ + p_).astype(bf16))

    wqkv_f = wqkv * anw[None, :]
    w1_f = w1 * fnw[None, :]
    w3_f = w3 * fnw[None, :]

    def ktile_major(wT, n):           # [HID, n] -> [128, KB_, n]
        return np.ascontiguousarray(
            wT.reshape(KB_, 128, n).transpose(1, 0, 2).astype(bf16))

    in_maps = []
    for c in range(NC):
        qrows = np.arange(JD * c, JD * (c + 1))
        krows = H * D + np.arange(D * c, D * (c + 1))
        vrows = (H + K) * D + np.arange(D * c, D * (c + 1))
        rows = np.concatenate([qrows, krows, vrows])
        w1T = w1_f[IS * c:IS * (c + 1)].T          # [HID, IS]
        w3T = w3_f[IS * c:IS * (c + 1)].T
        x2 = np.stack([x[512 * j + 64 * c: 512 * j + 64 * (c + 1)]
                       for j in range(4)])
        in_maps.append({
            "x_own": np.ascontiguousarray(x[TOK * c:TOK * (c + 1)]),
            "x2_own": np.ascontiguousarray(x2),
            "cosT": cosT_np, "sinT": sinT_np, "ident": ident_np,
            "masksI": masks_np,
            "wqkvR": ktile_major(wqkv_f[rows].T, JD + 2 * D),
            "woR": np.ascontiguousarray(
                wo[:, JD * c:JD * (c + 1)].T.reshape(QH, 128, HID)
                .transpose(1, 0, 2).astype(bf16)),
            "w1R": np.ascontiguousarray(
                w1T.reshape(KB_, 128, IT_, 128).transpose(2, 1, 0, 3)
                .astype(bf16)),
            "w3R": np.ascontiguousarray(
                w3T.reshape(KB_, 128, IT_, 128).transpose(2, 1, 0, 3)
                .astype(bf16)),
            "w2R": np.ascontiguousarray(
                w2[:, IS * c:IS * (c + 1)].T.reshape(IT_, 128, HID)
                .transpose(1, 0, 2).astype(bf16)),
        })
    return in_maps


def run(inputs, trace=False):
    """Returns (output, BassKernelResults)."""
    from concourse import bass_utils
    nc = _get_compiled()
    in_maps = _prep_inputs(inputs)
    res = bass_utils.run_bass_kernel_spmd(
        nc, in_maps, core_ids=list(range(NC)), trace=trace)
    out = np.empty((T, HID), np.float32)
    for c in range(NC):
        for j in range(4):
            out[512 * j + 64 * c: 512 * j + 64 * (c + 1)] = \
                res.results[c]["out_own"][j]
    return out, res


def kernel(**inputs):
    out, _ = run(inputs)
    return out
